# revision 25
# baseline (speedup 1.0000x reference)
"""Exp-kernel multivariate Hawkes process log-likelihood on 8 Trainium2 cores.

Data-parallel: one sequence (length L=2048) per core. The O(L^2) pairwise
exp-decay sum is computed per core as:
    W[i,j] = ln(alpha*beta)[e_i,e_j] - beta[e_i,e_j] * (t_i - t_j)
via a single K=20 matmul per 128x512 tile using a one-hot factorization
(host builds U (L,20) and V (L,20); W = U @ V^T), then exp(W) with a fused
row-sum on the scalar engine. The strict lower-triangular mask is applied by
adding -1e9 to the diagonal 128x512 PSUM slice before the exp. The clip
floor exp(-20) of the reference contributes < 1e-6 relative and is dropped
(validated numerically). The O(L*D) compensator runs on-device from
host-gathered (128,160) tiles; linear terms (mu*T, sum of column sums of
alpha) are folded into a host-side constant.
"""
import numpy as np

B, L, D = 8, 2048, 10
NB = L // 128            # 16 i-blocks of 128 rows
CH = 512                 # j-chunk width (one PSUM bank of fp32)
NCORES = 8

_CACHE = {}


def _build_nc():
    import concourse.bass as bass
    import concourse.bacc as bacc
    import concourse.tile as tile
    from concourse import mybir

    f32 = mybir.dt.float32
    Alu = mybir.AluOpType
    Act = mybir.ActivationFunctionType

    # Bacc (not raw Bass): its lowering legalizes sync waits for TRN2
    # (move_matmul_waits_to_ldweights + generate_event_semaphores split
    # multi-wait instructions, which walrus rejects otherwise)
    nc = bacc.Bacc()
    # uv cols 0:L = U^T, cols L:2L = V^T (both at base partition 0; single
    # DMA -> single PE wait per matmul)
    UV = nc.declare_dram_parameter("uv", [2 * D, 2 * L], f32, isOutput=False)
    # hostt cols: [0:NB]=mug, [NB:NB+D*NB]=argc (= ln(alpha) - beta*dt)
    HT = nc.declare_dram_parameter("hostt", [128, NB + D * NB], f32, isOutput=False)
    OUT = nc.declare_dram_parameter("out", [128, 1], f32, isOutput=True)

    with tile.TileContext(nc) as tc:
        with (
            tc.tile_pool(name="singles", bufs=1) as singles,
            tc.tile_pool(name="psum", bufs=2, space="PSUM") as psum,
            tc.tile_pool(name="scratch", bufs=2) as scratch,
        ):
            uv = singles.tile([2 * D, 2 * L], f32)
            nc.sync.dma_start(out=uv, in_=UV[:])
            ut = uv[:, 0:L]
            vt = uv[:, L:2 * L]
            ht = singles.tile([128, NB + D * NB], f32)
            nc.sync.dma_start(out=ht, in_=HT[:])
            mug = ht[:, 0:NB]
            argc = ht[:, NB:NB + D * NB]

            # iota[p, x] = x - p  (exact in fp32 for |values| < 2^24)
            iot = singles.tile([128, CH], f32)
            nc.gpsimd.iota(
                iot, pattern=[[1, CH]], channel_multiplier=-1,
                allow_small_or_imprecise_dtypes=True,
            )
            # masks[k][p, x] = -1e9 where x >= 128*k + p (j >= i), else 0
            masks = []
            for k in range(4):
                m = singles.tile([128, CH], f32, name=f"mask{k}")
                nc.vector.tensor_scalar(
                    out=m, in0=iot, scalar1=float(128 * k), scalar2=-1e9,
                    op0=Alu.is_ge, op1=Alu.mult,
                )
                masks.append(m)
            # dummy same-engine read so later mask consumers don't need a
            # DVE self-wait (walrus allows only one sync wait per instruction)
            dummy = singles.tile([128, 1], f32)
            nc.vector.tensor_copy(out=dummy, in_=masks[3][:, 0:1])

            pfull = singles.tile([128, NB], f32)
            pdiag = singles.tile([128, NB], f32)
            acc = singles.tile([128, NB + 1], f32)

            # blocks 0-3 have no full chunks: zero their pfull columns on ACT
            # (scale=0 copy of finite data) so pfull stays an ACT-only tile
            nc.scalar.activation(
                out=pfull[:, 0:4], in_=ht[:, 0:4], func=Act.Copy,
                bias=0.0, scale=0.0,
            )
            # compensator: sum_{d,i} exp(ln(alpha) - beta*(T - t_i)), fused row-sum
            exc = scratch.tile([128, D * NB], f32, tag="exc")
            nc.scalar.activation(
                out=exc, in_=argc, func=Act.Exp, accum_out=acc[:, NB:NB + 1],
            )

            for ib in range(NB):
                nch = ib // 4 + 1          # chunks of past events to cover
                prow = psum.tile([128, 4 * CH], f32, tag="prow")
                for c in range(nch):
                    nc.tensor.matmul(
                        prow[:, c * CH:(c + 1) * CH],
                        ut[:, ib * 128:(ib + 1) * 128],
                        vt[:, c * CH:(c + 1) * CH],
                        start=True, stop=True,
                    )
                # full chunks: exp + fused row-sum straight from PSUM (ACT-only reader)
                if nch > 1:
                    ex = scratch.tile([128, 3 * CH], f32, tag="ex")
                    nc.scalar.activation(
                        out=ex[:, :(nch - 1) * CH], in_=prow[:, :(nch - 1) * CH],
                        func=Act.Exp, accum_out=pfull[:, ib:ib + 1],
                    )
                # diagonal chunk: mask-add on DVE (PSUM -> SBUF; masked entries
                # -> -1e9 so exp underflows to 0; j<i entries have W <= ln(ab))
                exd = scratch.tile([128, CH], f32, tag="exd", bufs=NB)
                nc.vector.tensor_tensor(
                    out=exd, in0=prow[:, (nch - 1) * CH:nch * CH],
                    in1=masks[ib % 4], op=Alu.add,
                )
                sinkd = scratch.tile([128, CH], f32, tag="sinkd")
                nc.scalar.activation(
                    out=sinkd, in_=exd, func=Act.Exp,
                    accum_out=pdiag[:, ib:ib + 1],
                )

            inter = singles.tile([128, NB], f32)
            nc.vector.tensor_tensor(out=inter, in0=pfull, in1=pdiag, op=Alu.add)
            lam = singles.tile([128, NB], f32)
            nc.vector.tensor_tensor(out=lam, in0=inter, in1=mug, op=Alu.add)
            nc.scalar.activation(out=acc[:, 0:NB], in_=lam, func=Act.Ln)

            colsum = singles.tile([128, 1], f32)
            nc.vector.tensor_reduce(
                out=colsum, in_=acc, axis=mybir.AxisListType.X, op=Alu.add,
            )
            nc.sync.dma_start(out=OUT[:], in_=colsum)

    nc.finalize()  # runs Bacc.compile(): wait legalization + reg allocation
    return nc


def _softplus(x):
    return np.logaddexp(0.0, x.astype(np.float64))


def _host_prep(time_points, T, mu_raw, log_alpha, log_beta, event_types):
    """Per-core input tiles + additive host constants."""
    mu = _softplus(mu_raw).astype(np.float32)
    alpha = _softplus(log_alpha).astype(np.float32)
    beta = _softplus(log_beta).astype(np.float32)
    lnab = np.log(alpha.astype(np.float64) * beta.astype(np.float64)).astype(np.float32)
    colsumA = alpha.sum(0, dtype=np.float64)  # (D,)

    in_maps, consts = [], []
    for b in range(B):
        t = np.asarray(time_points[b], np.float32)
        e = np.asarray(event_types[b], np.int64)
        Tb = np.float64(T[b])

        U = np.empty((L, 2 * D), np.float32)
        U[:, :D] = lnab[e, :] - beta[e, :] * t[:, None]
        U[:, D:] = beta[e, :]
        E1 = np.zeros((L, D), np.float32)
        E1[np.arange(L), e] = 1.0
        V = np.concatenate([E1, E1 * t[:, None]], axis=1)

        # (p, ib) layout: i = 128*ib + p
        mug = mu[e].reshape(NB, 128).T.copy()                      # (128, NB)
        dt = (np.float32(Tb) - t).astype(np.float32)
        lna = np.log(alpha.astype(np.float64)).astype(np.float32)  # (D, D)
        argc = (lna[:, e] - beta[:, e] * dt[None, :]).astype(np.float32)  # (D, L)
        # (128, D*NB): col d*NB + ib <-> i = 128*ib + p
        argc = argc.reshape(D, NB, 128).transpose(2, 0, 1).reshape(128, D * NB).copy()

        const = -Tb * mu.sum(dtype=np.float64) - colsumA[e].sum()
        uv = np.concatenate([U.T, V.T], axis=1)            # (20, 2L)
        ht = np.concatenate([mug, argc], axis=1)           # (128, NB + D*NB)
        in_maps.append({
            "uv": np.ascontiguousarray(uv),
            "hostt": np.ascontiguousarray(ht),
        })
        consts.append(np.float32(const))
    return in_maps, consts


def kernel(**inputs):
    from concourse.bass_utils import run_bass_kernel_spmd

    if "nc" not in _CACHE:
        _CACHE["nc"] = _build_nc()
    nc = _CACHE["nc"]

    in_maps, consts = _host_prep(**inputs)
    res = run_bass_kernel_spmd(nc, in_maps, list(range(NCORES)))
    out = np.empty(B, np.float32)
    for b in range(B):
        colsum = res.results[b]["out"].reshape(128)
        out[b] = np.float32(np.float32(colsum.sum(dtype=np.float32)) + consts[b])
    return out


# revision 30
# speedup vs baseline: 1.4403x; 1.4403x over previous
"""Exp-kernel multivariate Hawkes process log-likelihood on 8 Trainium2 cores.

Data-parallel: one sequence (length L=2048) per core. The O(L^2) pairwise
exp-decay sum is computed per core as:
    W[i,j] = ln(alpha*beta)[e_i,e_j] - beta[e_i,e_j] * (t_i - t_j)
via a single K=20 matmul per 128x512 tile using a one-hot factorization
(host builds U (L,20) and V (L,20); W = U @ V^T), then exp(W) with a fused
row-sum on the scalar engine. The strict lower-triangular mask is applied by
adding -1e9 to the diagonal 128x512 PSUM slice before the exp. The clip
floor exp(-20) of the reference contributes < 1e-6 relative and is dropped
(validated numerically). The O(L*D) compensator runs on-device from
host-gathered (128,160) tiles; linear terms (mu*T, sum of column sums of
alpha) are folded into a host-side constant.
"""
import numpy as np

B, L, D = 8, 2048, 10
NB = L // 128            # 16 i-blocks of 128 rows
CH = 512                 # j-chunk width (one PSUM bank of fp32)
NCORES = 8

_CACHE = {}

# matmul operand dtype: float32r streams 1 col/cycle on the PE (vs 4 for
# float32, which lowers to 2 half-rate passes)
MM_DTYPE = "float32r"


def _build_nc():
    import concourse.bass as bass
    import concourse.bacc as bacc
    import concourse.tile as tile
    from concourse import mybir

    f32 = mybir.dt.float32
    Alu = mybir.AluOpType
    Act = mybir.ActivationFunctionType

    # Bacc (not raw Bass): its lowering legalizes sync waits for TRN2
    # (move_matmul_waits_to_ldweights + generate_event_semaphores split
    # multi-wait instructions, which walrus rejects otherwise)
    nc = bacc.Bacc()
    # uv cols 0:L = U^T, cols L:2L = V^T (both at base partition 0; single
    # DMA -> single PE wait per matmul)
    UV = nc.declare_dram_parameter("uv", [2 * D, 2 * L], f32, isOutput=False)
    # hostt cols: [0:NB]=mug, [NB:NB+D*NB]=argc (= ln(alpha) - beta*dt)
    HT = nc.declare_dram_parameter("hostt", [128, NB + D * NB], f32, isOutput=False)
    OUT = nc.declare_dram_parameter("out", [128, 1], f32, isOutput=True)

    with tile.TileContext(nc) as tc:
        with (
            tc.tile_pool(name="singles", bufs=1) as singles,
            tc.tile_pool(name="psum", bufs=2, space="PSUM") as psum,
            tc.tile_pool(name="scratch", bufs=2) as scratch,
        ):
            # two SBUF copies: walrus keys matmul precision off the memloc
            # dtype (AP bitcasts are ignored), so fp32 (diag) and fp32r
            # (off-diag) matmuls need separate tiles
            mmdt = getattr(mybir.dt, MM_DTYPE)
            uvf = singles.tile([2 * D, 2 * L], f32)
            nc.sync.dma_start(out=uvf, in_=UV[:])
            uvr = singles.tile([2 * D, 2 * L], mmdt, name="uvr")
            nc.sync.dma_start(out=uvr, in_=UV[:].bitcast(mmdt) if mmdt != f32 else UV[:])
            ht = singles.tile([128, NB + D * NB], f32)
            nc.sync.dma_start(out=ht, in_=HT[:])
            mug = ht[:, 0:NB]
            argc = ht[:, NB:NB + D * NB]

            # iota[p, x] = x - p  (exact in fp32 for |values| < 2^24)
            iot = singles.tile([128, CH], f32)
            nc.gpsimd.iota(
                iot, pattern=[[1, CH]], channel_multiplier=-1,
                allow_small_or_imprecise_dtypes=True,
            )
            # masks[k][p, x] = -1e9 where x >= 128*k + p (j >= i), else 0
            masks = []
            for k in range(4):
                m = singles.tile([128, CH], f32, name=f"mask{k}")
                nc.vector.tensor_scalar(
                    out=m, in0=iot, scalar1=float(128 * k), scalar2=-1e9,
                    op0=Alu.is_ge, op1=Alu.mult,
                )
                masks.append(m)
            # dummy same-engine read so later mask consumers don't need a
            # DVE self-wait (walrus allows only one sync wait per instruction)
            dummy = singles.tile([128, 1], f32)
            nc.vector.tensor_copy(out=dummy, in_=masks[3][:, 0:1])

            pfull = singles.tile([128, NB], f32)
            pdiag = singles.tile([128, NB], f32)
            acc = singles.tile([128, NB + 1], f32)

            # blocks 0-3 have no full chunks: zero their pfull columns on ACT
            # (scale=0 copy of finite data) so pfull stays an ACT-only tile
            nc.scalar.activation(
                out=pfull[:, 0:4], in_=ht[:, 0:4], func=Act.Copy,
                bias=0.0, scale=0.0,
            )
            # compensator: sum_{d,i} exp(ln(alpha) - beta*(T - t_i)), fused row-sum
            exc = scratch.tile([128, D * NB], f32, tag="exc")
            nc.scalar.activation(
                out=exc, in_=argc, func=Act.Exp, accum_out=acc[:, NB:NB + 1],
            )

            for ib in range(NB):
                nch = ib // 4 + 1          # chunks of past events to cover
                prow = psum.tile([128, 4 * CH], f32, tag="prow")
                for c in range(nch):
                    # diagonal chunk in full fp32 (dominant, undecayed terms);
                    # decayed off-diagonal chunks in fast fp32r
                    src = uvf if c == nch - 1 else uvr
                    nc.tensor.matmul(
                        prow[:, c * CH:(c + 1) * CH],
                        src[:, ib * 128:(ib + 1) * 128],
                        src[:, L + c * CH:L + (c + 1) * CH],
                        start=True, stop=True,
                    )
                # full chunks: exp + fused row-sum straight from PSUM (ACT-only reader)
                if nch > 1:
                    ex = scratch.tile([128, 3 * CH], f32, tag="ex")
                    nc.scalar.activation(
                        out=ex[:, :(nch - 1) * CH], in_=prow[:, :(nch - 1) * CH],
                        func=Act.Exp, accum_out=pfull[:, ib:ib + 1],
                    )
                # diagonal chunk: mask-add on DVE (PSUM -> SBUF; masked entries
                # -> -1e9 so exp underflows to 0; j<i entries have W <= ln(ab))
                exd = scratch.tile([128, CH], f32, tag="exd", bufs=NB)
                nc.vector.tensor_tensor(
                    out=exd, in0=prow[:, (nch - 1) * CH:nch * CH],
                    in1=masks[ib % 4], op=Alu.add,
                )
                sinkd = scratch.tile([128, CH], f32, tag="sinkd")
                nc.scalar.activation(
                    out=sinkd, in_=exd, func=Act.Exp,
                    accum_out=pdiag[:, ib:ib + 1],
                )

            inter = singles.tile([128, NB], f32)
            nc.vector.tensor_tensor(out=inter, in0=pfull, in1=pdiag, op=Alu.add)
            lam = singles.tile([128, NB], f32)
            nc.vector.tensor_tensor(out=lam, in0=inter, in1=mug, op=Alu.add)
            nc.scalar.activation(out=acc[:, 0:NB], in_=lam, func=Act.Ln)

            colsum = singles.tile([128, 1], f32)
            nc.vector.tensor_reduce(
                out=colsum, in_=acc, axis=mybir.AxisListType.X, op=Alu.add,
            )
            nc.sync.dma_start(out=OUT[:], in_=colsum)

    nc.finalize()  # runs Bacc.compile(): wait legalization + reg allocation
    return nc


def _softplus(x):
    return np.logaddexp(0.0, x.astype(np.float64))


def _host_prep(time_points, T, mu_raw, log_alpha, log_beta, event_types):
    """Per-core input tiles + additive host constants."""
    mu = _softplus(mu_raw).astype(np.float32)
    alpha = _softplus(log_alpha).astype(np.float32)
    beta = _softplus(log_beta).astype(np.float32)
    lnab = np.log(alpha.astype(np.float64) * beta.astype(np.float64)).astype(np.float32)
    colsumA = alpha.sum(0, dtype=np.float64)  # (D,)

    in_maps, consts = [], []
    for b in range(B):
        t = np.asarray(time_points[b], np.float32)
        e = np.asarray(event_types[b], np.int64)
        Tb = np.float64(T[b])

        U = np.empty((L, 2 * D), np.float32)
        U[:, :D] = lnab[e, :] - beta[e, :] * t[:, None]
        U[:, D:] = beta[e, :]
        E1 = np.zeros((L, D), np.float32)
        E1[np.arange(L), e] = 1.0
        V = np.concatenate([E1, E1 * t[:, None]], axis=1)

        # (p, ib) layout: i = 128*ib + p
        mug = mu[e].reshape(NB, 128).T.copy()                      # (128, NB)
        dt = (np.float32(Tb) - t).astype(np.float32)
        lna = np.log(alpha.astype(np.float64)).astype(np.float32)  # (D, D)
        argc = (lna[:, e] - beta[:, e] * dt[None, :]).astype(np.float32)  # (D, L)
        # (128, D*NB): col d*NB + ib <-> i = 128*ib + p
        argc = argc.reshape(D, NB, 128).transpose(2, 0, 1).reshape(128, D * NB).copy()

        const = -Tb * mu.sum(dtype=np.float64) - colsumA[e].sum()
        uv = np.concatenate([U.T, V.T], axis=1)            # (20, 2L)
        ht = np.concatenate([mug, argc], axis=1)           # (128, NB + D*NB)
        in_maps.append({
            "uv": np.ascontiguousarray(uv),
            "hostt": np.ascontiguousarray(ht),
        })
        consts.append(np.float32(const))
    return in_maps, consts


def kernel(**inputs):
    from concourse.bass_utils import run_bass_kernel_spmd

    if "nc" not in _CACHE:
        _CACHE["nc"] = _build_nc()
    nc = _CACHE["nc"]

    in_maps, consts = _host_prep(**inputs)
    res = run_bass_kernel_spmd(nc, in_maps, list(range(NCORES)))
    out = np.empty(B, np.float32)
    for b in range(B):
        colsum = res.results[b]["out"].reshape(128)
        out[b] = np.float32(np.float32(colsum.sum(dtype=np.float32)) + consts[b])
    return out


# revision 31
# speedup vs baseline: 1.6997x; 1.1802x over previous
"""Exp-kernel multivariate Hawkes process log-likelihood on 8 Trainium2 cores.

Data-parallel: one sequence (length L=2048) per core. The O(L^2) pairwise
exp-decay sum is computed per core as:
    W[i,j] = ln(alpha*beta)[e_i,e_j] - beta[e_i,e_j] * (t_i - t_j)
via K=20 matmuls using a one-hot factorization (host builds U (L,20) and
V (L,20); W = U @ V^T). Per 128-row i-block: the strictly-past "prefix"
columns j < 128*ib are unmasked and computed in float32r (1 col/cycle on
the PE; these terms are exponentially decayed so reduced precision is
harmless), while the 128-wide diagonal band gets full float32 (its
undecayed terms dominate lambda) plus a strict-lower-triangle mask.
exp() runs on the scalar engine with fused row-sum accumulation for the
prefix; the 16 diagonal bands' exps are batched into one SBUF tile and
reduced with a single vector op. The O(L*D) compensator runs on-device
from a host-gathered (128,160) tile with alpha folded into the exponent;
linear terms (mu*T, colsum(alpha) gather) fold into a host-side constant.
"""
import numpy as np

B, L, D = 8, 2048, 10
NB = L // 128            # 16 i-blocks of 128 rows
CH = 512                 # PSUM bank width in fp32
NCORES = 8

_CACHE = {}

# off-diagonal matmul dtype: float32r streams ~2-3x faster than float32
# (which lowers to 2 half-rate passes)
MM_DTYPE = "float32r"


def _build_nc():
    import concourse.bass as bass
    import concourse.bacc as bacc
    import concourse.tile as tile
    from concourse import mybir

    f32 = mybir.dt.float32
    Alu = mybir.AluOpType
    Act = mybir.ActivationFunctionType

    # Bacc (not raw Bass): its lowering legalizes sync waits for TRN2
    # (move_matmul_waits_to_ldweights + generate_event_semaphores)
    nc = bacc.Bacc()
    # uv cols 0:L = U^T, cols L:2L = V^T (both at base partition 0)
    UV = nc.declare_dram_parameter("uv", [2 * D, 2 * L], f32, isOutput=False)
    # hostt cols: [0:NB]=mug, [NB:NB+D*NB]=argc (= ln(alpha)-beta*dt),
    # [NB+D*NB : NB+D*NB+128] = iota (x - p), fp32
    HTW = NB + D * NB + 128
    HT = nc.declare_dram_parameter("hostt", [128, HTW], f32, isOutput=False)
    OUT = nc.declare_dram_parameter("out", [128, 1], f32, isOutput=True)

    with tile.TileContext(nc) as tc:
        with (
            tc.tile_pool(name="singles", bufs=1) as singles,
            tc.tile_pool(name="psum", bufs=2, space="PSUM") as psum,
            tc.tile_pool(name="scratch", bufs=2) as scratch,
        ):
            # two SBUF copies: walrus keys matmul precision off the memloc
            # dtype (AP bitcasts are ignored), so fp32 (band) and fp32r
            # (prefix) matmuls need separate tiles
            mmdt = getattr(mybir.dt, MM_DTYPE)
            uvf = singles.tile([2 * D, 2 * L], f32)
            nc.sync.dma_start(out=uvf, in_=UV[:])
            uvr = singles.tile([2 * D, 2 * L], mmdt, name="uvr")
            nc.sync.dma_start(out=uvr, in_=UV[:].bitcast(mmdt) if mmdt != f32 else UV[:])
            ht = singles.tile([128, HTW], f32)
            nc.sync.dma_start(out=ht, in_=HT[:])
            mug = ht[:, 0:NB]
            argc = ht[:, NB:NB + D * NB]
            iot = ht[:, NB + D * NB:NB + D * NB + 128]

            # band mask: -1e9 where x >= p (j >= i within the 128-wide band)
            mask = singles.tile([128, 128], f32)
            nc.vector.tensor_scalar(
                out=mask, in0=iot, scalar1=0.0, scalar2=-1e9,
                op0=Alu.is_ge, op1=Alu.mult,
            )

            pfull = singles.tile([128, NB], f32)
            acc = singles.tile([128, NB + 1], f32)
            # block 0 has no prefix: zero its pfull column on ACT
            # (scale=0 copy of finite data) so pfull stays an ACT-only tile
            nc.scalar.activation(
                out=pfull[:, 0:1], in_=ht[:, 0:1], func=Act.Copy,
                bias=0.0, scale=0.0,
            )
            # compensator: sum_{d,i} exp(ln(alpha) - beta*(T - t_i)), fused row-sum
            exc = scratch.tile([128, D * NB], f32, tag="exc")
            nc.scalar.activation(
                out=exc, in_=argc, func=Act.Exp, accum_out=acc[:, NB:NB + 1],
            )

            # masked band exponents collected across blocks, one column group
            # per block; a single DVE reduce produces all 16 band row-sums
            wband = singles.tile([128, NB * 128], f32)
            expb = singles.tile([128, NB * 128], f32)

            for ib in range(NB):
                pw = 128 * ib                 # prefix width (all j < band)
                prow = psum.tile([128, 4 * CH], f32, tag="prow")
                # prefix: unmasked, fp32r, 512-col bank-aligned pieces
                for k in range(0, pw, CH):
                    n = min(CH, pw - k)
                    nc.tensor.matmul(
                        prow[:, k:k + n],
                        uvr[:, ib * 128:(ib + 1) * 128],
                        uvr[:, L + k:L + k + n],
                        start=True, stop=True,
                    )
                # band: 128 cols at [pw, pw+128), full fp32
                nc.tensor.matmul(
                    prow[:, pw:pw + 128],
                    uvf[:, ib * 128:(ib + 1) * 128],
                    uvf[:, L + pw:L + pw + 128],
                    start=True, stop=True,
                )
                # prefix: exp + fused row-sum straight from PSUM
                if pw > 0:
                    ex = scratch.tile([128, 15 * 128], f32, tag="ex")
                    nc.scalar.activation(
                        out=ex[:, :pw], in_=prow[:, :pw],
                        func=Act.Exp, accum_out=pfull[:, ib:ib + 1],
                    )
                # band: mask on DVE (PSUM -> SBUF), exp on ACT into expb
                nc.vector.tensor_tensor(
                    out=wband[:, ib * 128:(ib + 1) * 128],
                    in0=prow[:, pw:pw + 128], in1=mask, op=Alu.add,
                )
                nc.scalar.activation(
                    out=expb[:, ib * 128:(ib + 1) * 128],
                    in_=wband[:, ib * 128:(ib + 1) * 128], func=Act.Exp,
                )

            pdiag = singles.tile([128, NB], f32)
            nc.vector.tensor_reduce(
                out=pdiag,
                in_=expb.rearrange("p (b x) -> p b x", b=NB),
                axis=mybir.AxisListType.X, op=Alu.add,
            )
            inter = singles.tile([128, NB], f32)
            nc.vector.tensor_tensor(out=inter, in0=pfull, in1=pdiag, op=Alu.add)
            lam = singles.tile([128, NB], f32)
            nc.vector.tensor_tensor(out=lam, in0=inter, in1=mug, op=Alu.add)
            nc.scalar.activation(out=acc[:, 0:NB], in_=lam, func=Act.Ln)

            colsum = singles.tile([128, 1], f32)
            nc.vector.tensor_reduce(
                out=colsum, in_=acc, axis=mybir.AxisListType.X, op=Alu.add,
            )
            nc.sync.dma_start(out=OUT[:], in_=colsum)

    nc.finalize()  # runs Bacc.compile(): wait legalization + reg allocation
    return nc


def _softplus(x):
    return np.logaddexp(0.0, x.astype(np.float64))


def _host_prep(time_points, T, mu_raw, log_alpha, log_beta, event_types):
    """Per-core input tiles + additive host constants."""
    mu = _softplus(mu_raw).astype(np.float32)
    alpha = _softplus(log_alpha).astype(np.float32)
    beta = _softplus(log_beta).astype(np.float32)
    lnab = np.log(alpha.astype(np.float64) * beta.astype(np.float64)).astype(np.float32)
    colsumA = alpha.sum(0, dtype=np.float64)  # (D,)
    lna = np.log(alpha.astype(np.float64)).astype(np.float32)  # (D, D)
    iot = (np.arange(128, dtype=np.float32)[None, :]
           - np.arange(128, dtype=np.float32)[:, None])        # (128,128) x - p

    in_maps, consts = [], []
    for b in range(B):
        t = np.asarray(time_points[b], np.float32)
        e = np.asarray(event_types[b], np.int64)
        Tb = np.float64(T[b])

        U = np.empty((L, 2 * D), np.float32)
        U[:, :D] = lnab[e, :] - beta[e, :] * t[:, None]
        U[:, D:] = beta[e, :]
        E1 = np.zeros((L, D), np.float32)
        E1[np.arange(L), e] = 1.0
        V = np.concatenate([E1, E1 * t[:, None]], axis=1)

        # (p, ib) layout: i = 128*ib + p
        mug = mu[e].reshape(NB, 128).T.copy()                      # (128, NB)
        dt = (np.float32(Tb) - t).astype(np.float32)
        argc = (lna[:, e] - beta[:, e] * dt[None, :]).astype(np.float32)  # (D, L)
        # (128, D*NB): col d*NB + ib <-> i = 128*ib + p
        argc = argc.reshape(D, NB, 128).transpose(2, 0, 1).reshape(128, D * NB).copy()

        const = -Tb * mu.sum(dtype=np.float64) - colsumA[e].sum()
        uv = np.concatenate([U.T, V.T], axis=1)            # (20, 2L)
        ht = np.concatenate([mug, argc, iot], axis=1)      # (128, NB+D*NB+128)
        in_maps.append({
            "uv": np.ascontiguousarray(uv),
            "hostt": np.ascontiguousarray(ht),
        })
        consts.append(np.float32(const))
    return in_maps, consts


def kernel(**inputs):
    from concourse.bass_utils import run_bass_kernel_spmd

    if "nc" not in _CACHE:
        _CACHE["nc"] = _build_nc()
    nc = _CACHE["nc"]

    in_maps, consts = _host_prep(**inputs)
    res = run_bass_kernel_spmd(nc, in_maps, list(range(NCORES)))
    out = np.empty(B, np.float32)
    for b in range(B):
        colsum = res.results[b]["out"].reshape(128)
        out[b] = np.float32(np.float32(colsum.sum(dtype=np.float32)) + consts[b])
    return out


# revision 38
# speedup vs baseline: 1.7805x; 1.0475x over previous
"""Exp-kernel multivariate Hawkes process log-likelihood on 8 Trainium2 cores.

Data-parallel: one sequence (length L=2048) per core. The O(L^2) pairwise
exp-decay sum is computed per core as:
    W[i,j] = ln(alpha*beta)[e_i,e_j] - beta[e_i,e_j] * (t_i - t_j)
via K=20 matmuls using a one-hot factorization (host builds U (L,20) and
V (L,20); W = U @ V^T). Per 128-row i-block: the strictly-past "prefix"
columns j < 128*ib are unmasked and computed in float32r (1 col/cycle on
the PE; these terms are exponentially decayed so reduced precision is
harmless), while the 128-wide diagonal band gets full float32 (its
undecayed terms dominate lambda) plus a strict-lower-triangle mask.
exp() runs on the scalar engine with fused row-sum accumulation for the
prefix; the 16 diagonal bands' exps are batched into one SBUF tile and
reduced with a single vector op. The O(L*D) compensator runs on-device
from a host-gathered (128,160) tile with alpha folded into the exponent;
linear terms (mu*T, colsum(alpha) gather) fold into a host-side constant.
"""
import numpy as np

B, L, D = 8, 2048, 10
NB = L // 128            # 16 i-blocks of 128 rows
CH = 512                 # PSUM bank width in fp32
NCORES = 8

_CACHE = {}

# All pairwise matmuls run in bf16 with a hi/lo compensated split: each
# fp32 factor x = x_hi + x_lo (bf16 parts); the K=20 contraction becomes
# K=60 stacked as hi*hi + hi*lo + lo*hi (the dropped lo*lo term is
# ~2^-18 relative). bf16 streams 1 col/cycle on the PE with fast weight
# loads vs 4 cycles/col for fp32 - and is ~5x more accurate here than
# single-pass float32r. The diagonal band additionally recenters times
# by a per-block offset (i-block == j-block on the band) to shrink
# products and hence the bf16 rounding error of the dominant terms.


def _build_nc():
    import concourse.bass as bass
    import concourse.bacc as bacc
    import concourse.tile as tile
    from concourse import mybir

    f32 = mybir.dt.float32
    Alu = mybir.AluOpType
    Act = mybir.ActivationFunctionType

    bf16 = mybir.dt.bfloat16
    # Bacc (not raw Bass): its lowering legalizes sync waits for TRN2
    # (move_matmul_waits_to_ldweights + generate_event_semaphores)
    nc = bacc.Bacc()
    # uv (60, 4L) bf16 column sections: [S_pre | M_pre | S_band | M_band],
    # each L wide. S = stationary stack [U_hi; U_hi; U_lo], M = moving
    # stack [V_hi; V_lo; V_hi]; band sections use per-block recentered t.
    UV = nc.declare_dram_parameter("uv", [3 * 2 * D, 4 * L], bf16, isOutput=False)
    # hostt cols: [0:NB]=mug, [NB:NB+D*NB]=argc (= ln(alpha)-beta*dt),
    # [NB+D*NB : NB+D*NB+128] = iota (x - p), fp32
    HTW = NB + D * NB + 128
    HT = nc.declare_dram_parameter("hostt", [128, HTW], f32, isOutput=False)
    OUT = nc.declare_dram_parameter("out", [128, 1], f32, isOutput=True)

    with tile.TileContext(nc) as tc:
        with (
            tc.tile_pool(name="singles", bufs=1) as singles,
            tc.tile_pool(name="psum", bufs=2, space="PSUM") as psum,
            tc.tile_pool(name="scratch", bufs=2) as scratch,
        ):
            uv = singles.tile([3 * 2 * D, 4 * L], bf16)
            nc.sync.dma_start(out=uv, in_=UV[:])
            ht = singles.tile([128, HTW], f32)
            nc.sync.dma_start(out=ht, in_=HT[:])
            mug = ht[:, 0:NB]
            argc = ht[:, NB:NB + D * NB]
            iot = ht[:, NB + D * NB:NB + D * NB + 128]

            # band mask: -1e9 where x >= p (j >= i within the 128-wide band)
            mask = singles.tile([128, 128], f32)
            nc.vector.tensor_scalar(
                out=mask, in0=iot, scalar1=0.0, scalar2=-1e9,
                op0=Alu.is_ge, op1=Alu.mult,
            )

            pfull = singles.tile([128, NB], f32)
            acc = singles.tile([128, NB + 1], f32)
            # block 0 has no prefix: zero its pfull column on ACT
            # (scale=0 copy of finite data) so pfull stays an ACT-only tile
            nc.scalar.activation(
                out=pfull[:, 0:1], in_=ht[:, 0:1], func=Act.Copy,
                bias=0.0, scale=0.0,
            )
            # compensator: sum_{d,i} exp(ln(alpha) - beta*(T - t_i)), fused row-sum
            exc = scratch.tile([128, D * NB], f32, tag="exc")
            nc.scalar.activation(
                out=exc, in_=argc, func=Act.Exp, accum_out=acc[:, NB:NB + 1],
            )

            # masked band exponents collected across blocks, one column group
            # per block; a single DVE reduce produces all 16 band row-sums
            wband = singles.tile([128, NB * 128], f32)
            expb = singles.tile([128, NB * 128], f32)

            for ib in range(NB):
                pw = 128 * ib                 # prefix width (all j < band)
                prow = psum.tile([128, 4 * CH], f32, tag="prow")
                # prefix: unmasked, 512-col bank-aligned pieces
                for k in range(0, pw, CH):
                    n = min(CH, pw - k)
                    nc.tensor.matmul(
                        prow[:, k:k + n],
                        uv[:, ib * 128:(ib + 1) * 128],
                        uv[:, L + k:L + k + n],
                        start=True, stop=True,
                    )
                # band: 128 cols at [pw, pw+128), recentered sections
                nc.tensor.matmul(
                    prow[:, pw:pw + 128],
                    uv[:, 2 * L + ib * 128:2 * L + (ib + 1) * 128],
                    uv[:, 3 * L + pw:3 * L + pw + 128],
                    start=True, stop=True,
                )
                # prefix: exp + fused row-sum straight from PSUM
                if pw > 0:
                    ex = scratch.tile([128, 15 * 128], f32, tag="ex")
                    nc.scalar.activation(
                        out=ex[:, :pw], in_=prow[:, :pw],
                        func=Act.Exp, accum_out=pfull[:, ib:ib + 1],
                    )
                # band: mask on DVE (PSUM -> SBUF), exp on ACT into expb
                nc.vector.tensor_tensor(
                    out=wband[:, ib * 128:(ib + 1) * 128],
                    in0=prow[:, pw:pw + 128], in1=mask, op=Alu.add,
                )
                nc.scalar.activation(
                    out=expb[:, ib * 128:(ib + 1) * 128],
                    in_=wband[:, ib * 128:(ib + 1) * 128], func=Act.Exp,
                )

            pdiag = singles.tile([128, NB], f32)
            nc.vector.tensor_reduce(
                out=pdiag,
                in_=expb.rearrange("p (b x) -> p b x", b=NB),
                axis=mybir.AxisListType.X, op=Alu.add,
            )
            inter = singles.tile([128, NB], f32)
            nc.vector.tensor_tensor(out=inter, in0=pfull, in1=pdiag, op=Alu.add)
            lam = singles.tile([128, NB], f32)
            nc.vector.tensor_tensor(out=lam, in0=inter, in1=mug, op=Alu.add)
            nc.scalar.activation(out=acc[:, 0:NB], in_=lam, func=Act.Ln)

            colsum = singles.tile([128, 1], f32)
            nc.vector.tensor_reduce(
                out=colsum, in_=acc, axis=mybir.AxisListType.X, op=Alu.add,
            )
            nc.sync.dma_start(out=OUT[:], in_=colsum)

    nc.finalize()  # runs Bacc.compile(): wait legalization + reg allocation
    return nc


def _softplus(x):
    return np.logaddexp(0.0, x.astype(np.float64))


def _bf16(x):
    import jax.numpy as jnp
    return np.asarray(jnp.asarray(x, jnp.bfloat16))


def _hilo_stacks(U, V):
    """K=60 bf16 compensated stacks: S = [U_hi;U_hi;U_lo], M = [V_hi;V_lo;V_hi].
    Returns (S^T, M^T) as (60, L) float arrays in bf16 values."""
    Uh = _bf16(U)
    Ul = _bf16(U - Uh.astype(np.float32))
    Vh = _bf16(V)
    Vl = _bf16(V - Vh.astype(np.float32))
    S = np.concatenate([Uh, Uh, Ul], axis=1)   # (L, 60)
    M = np.concatenate([Vh, Vl, Vh], axis=1)   # (L, 60)
    return S.T.copy(), M.T.copy()


def _host_prep(time_points, T, mu_raw, log_alpha, log_beta, event_types):
    """Per-core input tiles + additive host constants."""
    mu = _softplus(mu_raw).astype(np.float32)
    alpha = _softplus(log_alpha).astype(np.float32)
    beta = _softplus(log_beta).astype(np.float32)
    lnab = np.log(alpha.astype(np.float64) * beta.astype(np.float64)).astype(np.float32)
    colsumA = alpha.sum(0, dtype=np.float64)  # (D,)
    lna = np.log(alpha.astype(np.float64)).astype(np.float32)  # (D, D)
    iot = (np.arange(128, dtype=np.float32)[None, :]
           - np.arange(128, dtype=np.float32)[:, None])        # (128,128) x - p

    in_maps, consts = [], []
    for b in range(B):
        t = np.asarray(time_points[b], np.float32)
        e = np.asarray(event_types[b], np.int64)
        Tb = np.float64(T[b])

        U = np.empty((L, 2 * D), np.float32)
        U[:, :D] = lnab[e, :] - beta[e, :] * t[:, None]
        U[:, D:] = beta[e, :]
        E1 = np.zeros((L, D), np.float32)
        E1[np.arange(L), e] = 1.0
        V = np.concatenate([E1, E1 * t[:, None]], axis=1)
        # band variants: recenter t by each block's mean time (band matmuls
        # pair i and j from the same 128-block, so offsets cancel exactly)
        cblk = t.reshape(NB, 128).mean(axis=1).astype(np.float32)
        tb = (t - np.repeat(cblk, 128)).astype(np.float32)
        Ub = np.empty((L, 2 * D), np.float32)
        Ub[:, :D] = lnab[e, :] - beta[e, :] * tb[:, None]
        Ub[:, D:] = beta[e, :]
        Vb = np.concatenate([E1, E1 * tb[:, None]], axis=1)
        Sp, Mp = _hilo_stacks(U, V)
        Sb, Mb = _hilo_stacks(Ub, Vb)

        # (p, ib) layout: i = 128*ib + p
        mug = mu[e].reshape(NB, 128).T.copy()                      # (128, NB)
        dt = (np.float32(Tb) - t).astype(np.float32)
        argc = (lna[:, e] - beta[:, e] * dt[None, :]).astype(np.float32)  # (D, L)
        # (128, D*NB): col d*NB + ib <-> i = 128*ib + p
        argc = argc.reshape(D, NB, 128).transpose(2, 0, 1).reshape(128, D * NB).copy()

        const = -Tb * mu.sum(dtype=np.float64) - colsumA[e].sum()
        uv = _bf16(np.concatenate([Sp, Mp, Sb, Mb], axis=1))  # (60, 4L) bf16
        ht = np.concatenate([mug, argc, iot], axis=1)      # (128, NB+D*NB+128)
        in_maps.append({
            "uv": np.ascontiguousarray(uv),
            "hostt": np.ascontiguousarray(ht),
        })
        consts.append(np.float32(const))
    return in_maps, consts


def kernel(**inputs):
    from concourse.bass_utils import run_bass_kernel_spmd

    if "nc" not in _CACHE:
        _CACHE["nc"] = _build_nc()
    nc = _CACHE["nc"]

    in_maps, consts = _host_prep(**inputs)
    res = run_bass_kernel_spmd(nc, in_maps, list(range(NCORES)))
    out = np.empty(B, np.float32)
    for b in range(B):
        colsum = res.results[b]["out"].reshape(128)
        out[b] = np.float32(np.float32(colsum.sum(dtype=np.float32)) + consts[b])
    return out


# revision 40
# speedup vs baseline: 1.8270x; 1.0261x over previous
"""Exp-kernel multivariate Hawkes process log-likelihood on 8 Trainium2 cores.

Data-parallel: one sequence (length L=2048) per core. The O(L^2) pairwise
exp-decay sum is computed per core as:
    W[i,j] = ln(alpha*beta)[e_i,e_j] - beta[e_i,e_j] * (t_i - t_j)
via K=20 matmuls using a one-hot factorization (host builds U (L,20) and
V (L,20); W = U @ V^T). Per 128-row i-block: the strictly-past "prefix"
columns j < 128*ib are unmasked and computed in float32r (1 col/cycle on
the PE; these terms are exponentially decayed so reduced precision is
harmless), while the 128-wide diagonal band gets full float32 (its
undecayed terms dominate lambda) plus a strict-lower-triangle mask.
exp() runs on the scalar engine with fused row-sum accumulation for the
prefix; the 16 diagonal bands' exps are batched into one SBUF tile and
reduced with a single vector op. The O(L*D) compensator runs on-device
from a host-gathered (128,160) tile with alpha folded into the exponent;
linear terms (mu*T, colsum(alpha) gather) fold into a host-side constant.
"""
import numpy as np

B, L, D = 8, 2048, 10
NB = L // 128            # 16 i-blocks of 128 rows
CH = 512                 # PSUM bank width in fp32
NCORES = 8

_CACHE = {}

# All pairwise matmuls run in bf16 with a hi/lo compensated split: each
# fp32 factor x = x_hi + x_lo (bf16 parts); the K=20 contraction becomes
# K=60 stacked as hi*hi + hi*lo + lo*hi (the dropped lo*lo term is
# ~2^-18 relative). bf16 streams 1 col/cycle on the PE with fast weight
# loads vs 4 cycles/col for fp32 - and is ~5x more accurate here than
# single-pass float32r. The diagonal band additionally recenters times
# by a per-block offset (i-block == j-block on the band) to shrink
# products and hence the bf16 rounding error of the dominant terms.


def _build_nc():
    import concourse.bass as bass
    import concourse.bacc as bacc
    import concourse.tile as tile
    from concourse import mybir

    f32 = mybir.dt.float32
    Alu = mybir.AluOpType
    Act = mybir.ActivationFunctionType

    bf16 = mybir.dt.bfloat16
    # Bacc (not raw Bass): its lowering legalizes sync waits for TRN2
    # (move_matmul_waits_to_ldweights + generate_event_semaphores)
    nc = bacc.Bacc()
    # uv (60, 4L) bf16 column sections: [S_pre | M_pre | S_band | M_band],
    # each L wide. S = stationary stack [U_hi; U_hi; U_lo], M = moving
    # stack [V_hi; V_lo; V_hi]; band sections use per-block recentered t.
    UV = nc.declare_dram_parameter("uv", [3 * 2 * D, 4 * L], bf16, isOutput=False)
    # hostt cols: [0:NB]=mug, [NB:NB+D*NB]=argc (= ln(alpha)-beta*dt),
    # [NB+D*NB : NB+D*NB+128] = iota (x - p), fp32
    HTW = NB + D * NB + 128
    HT = nc.declare_dram_parameter("hostt", [128, HTW], f32, isOutput=False)
    OUT = nc.declare_dram_parameter("out", [128, 1], f32, isOutput=True)

    with tile.TileContext(nc) as tc:
        with (
            tc.tile_pool(name="singles", bufs=1) as singles,
            tc.tile_pool(name="psum", bufs=2, space="PSUM") as psum,
            tc.tile_pool(name="scratch", bufs=2) as scratch,
        ):
            uv = singles.tile([3 * 2 * D, 4 * L], bf16)
            # split the load so early blocks' sections land first
            nc.sync.dma_start(out=uv[:, 0:2 * L], in_=UV[:, 0:2 * L])
            nc.sync.dma_start(out=uv[:, 2 * L:4 * L], in_=UV[:, 2 * L:4 * L])
            ht = singles.tile([128, HTW], f32)
            nc.sync.dma_start(out=ht, in_=HT[:])
            mug = ht[:, 0:NB]
            argc = ht[:, NB:NB + D * NB]
            iot = ht[:, NB + D * NB:NB + D * NB + 128]

            # band mask: -1e9 where x >= p (j >= i within the 128-wide band)
            mask = singles.tile([128, 128], f32)
            nc.vector.tensor_scalar(
                out=mask, in0=iot, scalar1=0.0, scalar2=-1e9,
                op0=Alu.is_ge, op1=Alu.mult,
            )

            pfull = singles.tile([128, NB], f32)
            acc = singles.tile([128, NB + 1], f32)
            # block 0 has no prefix: zero its pfull column on ACT
            # (scale=0 copy of finite data) so pfull stays an ACT-only tile
            nc.scalar.activation(
                out=pfull[:, 0:1], in_=ht[:, 0:1], func=Act.Copy,
                bias=0.0, scale=0.0,
            )
            # compensator: sum_{d,i} exp(ln(alpha) - beta*(T - t_i)), fused row-sum
            exc = scratch.tile([128, D * NB], f32, tag="exc")
            nc.scalar.activation(
                out=exc, in_=argc, func=Act.Exp, accum_out=acc[:, NB:NB + 1],
            )

            # masked band exponents collected across blocks, one column group
            # per block; a single DVE reduce produces all 16 band row-sums
            wband = singles.tile([128, NB * 128], f32)
            expb = singles.tile([128, NB * 128], f32)

            for ib in range(NB):
                pw = 128 * ib                 # prefix width (all j < band)
                prow = psum.tile([128, 4 * CH], f32, tag="prow")
                # prefix: unmasked, 512-col bank-aligned pieces
                for k in range(0, pw, CH):
                    n = min(CH, pw - k)
                    nc.tensor.matmul(
                        prow[:, k:k + n],
                        uv[:, ib * 128:(ib + 1) * 128],
                        uv[:, L + k:L + k + n],
                        start=True, stop=True,
                    )
                # band: 128 cols at [pw, pw+128), recentered sections
                nc.tensor.matmul(
                    prow[:, pw:pw + 128],
                    uv[:, 2 * L + ib * 128:2 * L + (ib + 1) * 128],
                    uv[:, 3 * L + pw:3 * L + pw + 128],
                    start=True, stop=True,
                )
                # prefix: exp + fused row-sum straight from PSUM
                if pw > 0:
                    ex = scratch.tile([128, 15 * 128], f32, tag="ex")
                    nc.scalar.activation(
                        out=ex[:, :pw], in_=prow[:, :pw],
                        func=Act.Exp, accum_out=pfull[:, ib:ib + 1],
                    )
                # band: mask on DVE (PSUM -> SBUF); exp batched 4 bands/op
                nc.vector.tensor_tensor(
                    out=wband[:, ib * 128:(ib + 1) * 128],
                    in0=prow[:, pw:pw + 128], in1=mask, op=Alu.add,
                )
                if ib % 4 == 3:
                    g = ib // 4
                    nc.scalar.activation(
                        out=expb[:, g * 512:(g + 1) * 512],
                        in_=wband[:, g * 512:(g + 1) * 512], func=Act.Exp,
                    )

            pdiag = singles.tile([128, NB], f32)
            for g in range(4):
                nc.vector.tensor_reduce(
                    out=pdiag[:, 4 * g:4 * (g + 1)],
                    in_=expb[:, g * 512:(g + 1) * 512].rearrange(
                        "p (b x) -> p b x", b=4),
                    axis=mybir.AxisListType.X, op=Alu.add,
                )
            inter = singles.tile([128, NB], f32)
            nc.vector.tensor_tensor(out=inter, in0=pfull, in1=pdiag, op=Alu.add)
            lam = singles.tile([128, NB], f32)
            nc.vector.tensor_tensor(out=lam, in0=inter, in1=mug, op=Alu.add)
            nc.scalar.activation(out=acc[:, 0:NB], in_=lam, func=Act.Ln)

            colsum = singles.tile([128, 1], f32)
            nc.vector.tensor_reduce(
                out=colsum, in_=acc, axis=mybir.AxisListType.X, op=Alu.add,
            )
            nc.sync.dma_start(out=OUT[:], in_=colsum)

    nc.finalize()  # runs Bacc.compile(): wait legalization + reg allocation
    return nc


def _softplus(x):
    return np.logaddexp(0.0, x.astype(np.float64))


def _bf16(x):
    import jax.numpy as jnp
    return np.asarray(jnp.asarray(x, jnp.bfloat16))


def _hilo_stacks(U, V):
    """K=60 bf16 compensated stacks: S = [U_hi;U_hi;U_lo], M = [V_hi;V_lo;V_hi].
    Returns (S^T, M^T) as (60, L) float arrays in bf16 values."""
    Uh = _bf16(U)
    Ul = _bf16(U - Uh.astype(np.float32))
    Vh = _bf16(V)
    Vl = _bf16(V - Vh.astype(np.float32))
    S = np.concatenate([Uh, Uh, Ul], axis=1)   # (L, 60)
    M = np.concatenate([Vh, Vl, Vh], axis=1)   # (L, 60)
    return S.T.copy(), M.T.copy()


def _host_prep(time_points, T, mu_raw, log_alpha, log_beta, event_types):
    """Per-core input tiles + additive host constants."""
    mu = _softplus(mu_raw).astype(np.float32)
    alpha = _softplus(log_alpha).astype(np.float32)
    beta = _softplus(log_beta).astype(np.float32)
    lnab = np.log(alpha.astype(np.float64) * beta.astype(np.float64)).astype(np.float32)
    colsumA = alpha.sum(0, dtype=np.float64)  # (D,)
    lna = np.log(alpha.astype(np.float64)).astype(np.float32)  # (D, D)
    iot = (np.arange(128, dtype=np.float32)[None, :]
           - np.arange(128, dtype=np.float32)[:, None])        # (128,128) x - p

    in_maps, consts = [], []
    for b in range(B):
        t = np.asarray(time_points[b], np.float32)
        e = np.asarray(event_types[b], np.int64)
        Tb = np.float64(T[b])

        U = np.empty((L, 2 * D), np.float32)
        U[:, :D] = lnab[e, :] - beta[e, :] * t[:, None]
        U[:, D:] = beta[e, :]
        E1 = np.zeros((L, D), np.float32)
        E1[np.arange(L), e] = 1.0
        V = np.concatenate([E1, E1 * t[:, None]], axis=1)
        # band variants: recenter t by each block's mean time (band matmuls
        # pair i and j from the same 128-block, so offsets cancel exactly)
        cblk = t.reshape(NB, 128).mean(axis=1).astype(np.float32)
        tb = (t - np.repeat(cblk, 128)).astype(np.float32)
        Ub = np.empty((L, 2 * D), np.float32)
        Ub[:, :D] = lnab[e, :] - beta[e, :] * tb[:, None]
        Ub[:, D:] = beta[e, :]
        Vb = np.concatenate([E1, E1 * tb[:, None]], axis=1)
        Sp, Mp = _hilo_stacks(U, V)
        Sb, Mb = _hilo_stacks(Ub, Vb)

        # (p, ib) layout: i = 128*ib + p
        mug = mu[e].reshape(NB, 128).T.copy()                      # (128, NB)
        dt = (np.float32(Tb) - t).astype(np.float32)
        argc = (lna[:, e] - beta[:, e] * dt[None, :]).astype(np.float32)  # (D, L)
        # (128, D*NB): col d*NB + ib <-> i = 128*ib + p
        argc = argc.reshape(D, NB, 128).transpose(2, 0, 1).reshape(128, D * NB).copy()

        const = -Tb * mu.sum(dtype=np.float64) - colsumA[e].sum()
        uv = _bf16(np.concatenate([Sp, Mp, Sb, Mb], axis=1))  # (60, 4L) bf16
        ht = np.concatenate([mug, argc, iot], axis=1)      # (128, NB+D*NB+128)
        in_maps.append({
            "uv": np.ascontiguousarray(uv),
            "hostt": np.ascontiguousarray(ht),
        })
        consts.append(np.float32(const))
    return in_maps, consts


def kernel(**inputs):
    from concourse.bass_utils import run_bass_kernel_spmd

    if "nc" not in _CACHE:
        _CACHE["nc"] = _build_nc()
    nc = _CACHE["nc"]

    in_maps, consts = _host_prep(**inputs)
    res = run_bass_kernel_spmd(nc, in_maps, list(range(NCORES)))
    out = np.empty(B, np.float32)
    for b in range(B):
        colsum = res.results[b]["out"].reshape(128)
        out[b] = np.float32(np.float32(colsum.sum(dtype=np.float32)) + consts[b])
    return out
